# revision 3
# baseline (speedup 1.0000x reference)
"""Trainium2 Bass kernel: batched soft 3-SAT circuit evaluation.

Computes out[b, c] = 1 - prod_k z[c,k],  z = (sign>0 ? 1-x : x)[idx],
x = sigmoid(emb[0]) — every batch row is identical (input_idx is all
zeros and the embedding has a single row; jnp.take clamps OOB indices,
so the result provably never depends on input_idx).

Sharding: clauses are split across the 8 NeuronCores (5250 each); each
core computes its clause outputs once and broadcast-writes all 1024
batch rows of its column block.  Host-side work is limited to index
layout prep (fold sign into a combined table index, pad, wrap into the
16-partition GPSIMD gather layout) and concatenating per-core outputs.

Device per core:
  1. DMA idx (int16 [128,126]) + broadcast-DMA emb row -> [128, NV]
  2. ACT: table[:, :NV] = sigmoid(-w) = 1-x ; table[:, NV:] = sigmoid(w)
  3. GPSIMD ap_gather: z[128, 2016] = table[idx'] (8 Q7 cores, each
     handling its 16-partition group's 2016 literals = 672 clauses)
  4. DVE: r = 1 - z0*z1*z2  ([128, 672] clause results; partition group
     g holds clauses [672g, 672g+672))
  5. One DMA writes out[1024, 5376]: for each partition p=16g+j, its 672
     clause results are written to rows j+16b (b=0..63) at cols [672g,
     672g+672) — a stride-0 source dim replicates the row 64x.
"""

import numpy as np

NV = 10000
C_TOTAL = 42000
KLIT = 3
B = 1024
NCORES = 8
C_CORE = C_TOTAL // NCORES     # 5250
GROUPS = 8                     # Q7 cores / 16-partition groups
CG = 672                       # padded clauses per group (8*672 = 5376)
LITG = CG * KLIT               # 2016 literals per group
IDX_COLS = LITG // 16          # 126
C_PAD = GROUPS * CG            # 5376 padded output columns per core

_CACHE = {}


def _build(strategy="bigdma"):
    import concourse.bass as bass
    import concourse.tile as tile
    from concourse import bacc, mybir
    from contextlib import ExitStack

    f32 = mybir.dt.float32
    AF = mybir.ActivationFunctionType
    OP = mybir.AluOpType

    nc = bacc.Bacc("TRN2", target_bir_lowering=False, debug=False,
                   num_devices=NCORES)
    emb_d = nc.dram_tensor("emb", [1, NV], f32, kind="ExternalInput")
    idx_d = nc.dram_tensor("idxw", [128, IDX_COLS], mybir.dt.int16,
                           kind="ExternalInput")
    out_d = nc.dram_tensor("out", [B, C_PAD], f32, kind="ExternalOutput")

    with tile.TileContext(nc) as tc, ExitStack() as ctx:
        pool = ctx.enter_context(tc.tile_pool(name="main", bufs=1))

        idx_sb = pool.tile([128, IDX_COLS], mybir.dt.int16)
        nc.sync.dma_start(out=idx_sb[:], in_=idx_d[:, :])

        raw = pool.tile([128, NV], f32)
        nc.sync.dma_start(
            out=raw[:],
            in_=bass.AP(tensor=emb_d, offset=0, ap=[[0, 128], [1, NV]]))

        tab = pool.tile([128, 2 * NV], f32)
        nc.scalar.activation(tab[:, 0:NV], raw[:], AF.Sigmoid, scale=-1.0)
        nc.scalar.activation(tab[:, NV:2 * NV], raw[:], AF.Sigmoid)

        z = pool.tile([128, LITG], f32)
        nc.gpsimd.ap_gather(z[:], tab[:], idx_sb[:], channels=128,
                            num_elems=2 * NV, d=1, num_idxs=LITG)

        p01 = pool.tile([128, CG], f32)
        nc.vector.tensor_tensor(p01[:], z[:, 0:LITG:3], z[:, 1:LITG:3],
                                OP.mult)
        q = pool.tile([128, CG], f32)
        # q = (p01 * -1) * z2 = -(z0 z1 z2)
        nc.vector.scalar_tensor_tensor(q[:], p01[:], -1.0, z[:, 2:LITG:3],
                                       OP.mult, OP.mult)
        r = pool.tile([128, CG], f32)
        nc.vector.tensor_scalar_add(r[:], q[:], 1.0)

        rap = r[:]
        pstride = rap.ap[0][0]
        if strategy == "bigdma":
            # One DMA per 16-partition group g (DMA APs are limited to 3
            # dims): src = [16 partitions, 64 stride-0 reps, 672 cols],
            # dst rows j+16b at cols [672g, 672g+672).
            for g in range(GROUPS):
                src = bass.AP(tensor=rap.tensor,
                              offset=rap.offset + pstride * 16 * g,
                              ap=[[pstride, 16], [0, 64], [1, CG]])
                dst = bass.AP(tensor=out_d, offset=g * CG,
                              ap=[[C_PAD, 16], [16 * C_PAD, 64], [1, CG]])
                nc.sync.dma_start(out=dst, in_=src)
        else:
            # plain-AP fallback: assemble one row, log-double to 128
            # partitions, then 8 row-block DMAs.
            brow = pool.tile([128, C_PAD], f32)
            src0 = bass.AP(tensor=rap.tensor, offset=rap.offset,
                           ap=[[pstride * 16, 8], [1, CG]])
            nc.sync.dma_start(out=brow[0:1, :], in_=src0)
            p = 1
            while p < 128:
                nc.sync.dma_start(out=brow[p:2 * p, :], in_=brow[0:p, :])
                p *= 2
            for blk in range(8):
                nc.sync.dma_start(
                    out=out_d[blk * 128:(blk + 1) * 128, :], in_=brow[:])
    nc.compile()
    return nc


def _prep_indices(clause_idx, clause_sign):
    """Per-core wrapped int16 combined-index arrays [128, IDX_COLS]."""
    idx2 = clause_idx.astype(np.int32) + NV * (clause_sign <= 0.0)
    idx2 = idx2.astype(np.int16)                    # < 20000, fits
    per_core = []
    for c in range(NCORES):
        lit = idx2[c * C_CORE:(c + 1) * C_CORE].reshape(-1)   # [15750]
        buf = np.zeros(GROUPS * LITG, dtype=np.int16)         # pad -> idx 0
        buf[:lit.size] = lit
        # group g literals i live at partition 16g + i%16, col i//16
        w = (buf.reshape(GROUPS, IDX_COLS, 16)
                .transpose(0, 2, 1)
                .reshape(128, IDX_COLS))
        per_core.append(np.ascontiguousarray(w))
    return per_core


def _ensure_ntff_hook():
    """The agent image lacks antenv.axon_hooks; synthesize it so
    run_bass_kernel_spmd(trace=True) can capture NTFF profiles."""
    import sys, types
    try:
        from antenv import axon_hooks  # noqa: F401
        return
    except ImportError:
        pass
    m = types.ModuleType("antenv.axon_hooks")
    _hook = [None]
    m.set_axon_ntff_profile_hook = lambda h: _hook.__setitem__(0, h)
    m.get_axon_ntff_profile_hook = lambda: _hook[0]
    sys.modules["antenv.axon_hooks"] = m
    import antenv
    antenv.axon_hooks = m
    from trn_agent_boot.trn_boot import _ntff_profile_via_ctypes
    m.set_axon_ntff_profile_hook(
        _ntff_profile_via_ctypes("/opt/axon/libaxon_pjrt.so"))


def _run(emb, idx_cores, trace=False):
    from concourse.bass_utils import run_bass_kernel_spmd
    if trace:
        _ensure_ntff_hook()
    key = "prog"
    if key not in _CACHE:
        _CACHE[key] = _build()
    nc = _CACHE[key]
    in_maps = [{"emb": emb, "idxw": idx_cores[c]} for c in range(NCORES)]
    res = run_bass_kernel_spmd(nc, in_maps, list(range(NCORES)),
                               trace=trace)
    return res


def kernel(input_idx=None, emb_weight=None, clause_idx=None,
           clause_sign=None, _trace=False, _want_results=False):
    emb = np.ascontiguousarray(np.asarray(emb_weight, dtype=np.float32))
    cidx = np.asarray(clause_idx, dtype=np.int32)
    csgn = np.asarray(clause_sign, dtype=np.float32)
    idx_cores = _prep_indices(cidx, csgn)
    res = _run(emb, idx_cores, trace=_trace)
    full = np.empty((B, C_TOTAL), dtype=np.float32)
    for c in range(NCORES):
        full[:, c * C_CORE:(c + 1) * C_CORE] = \
            res.results[c]["out"][:, :C_CORE]
    if _want_results:
        return full, res
    return full


# revision 9
# speedup vs baseline: 1.7280x; 1.7280x over previous
"""Trainium2 Bass kernel: batched soft 3-SAT circuit evaluation.

out[b, c] = 1 - prod_k z[c,k],  z = (sign>0 ? 1-x : x)[idx],
x = sigmoid(emb[0]).  Every batch row is identical (input_idx is all
zeros, the embedding has a single row, and jnp.take clamps OOB), so the
device computes each clause result once and broadcast-writes the rows.

Sharding: clauses split across 8 NeuronCores (5250 each, padded 5376).
Host work is index-layout prep only (fold sign into a combined table
index, pad, order literals chunk-major, wrap into the 16-partition
GPSIMD gather layout) plus concatenation of per-core outputs.

Per-core device pipeline (H = 2 column chunks):
  prologue: broadcast emb row into [128, NV] (two DMAs on the two HWDGE
    rings), then per half: ACT sigmoid -> x table half; DVE (x*-1)+1 ->
    1-x table half.  Combined table [128, 2*NV].
  per chunk h (2688 output cols each):
    - GPSIMD ap_gather: z[128, 1008] from the table (8 Q7 groups x
      336 clauses each)
    - DVE: r = 1 - z0*z1*z2  [128, 336]
    - assemble row chunk to DRAM scratch (one partition per group),
      broadcast-read it back into B[128, 2688] (both on SWDGE),
    - 8 row-block DMAs B -> out[128b:128b+128, 2688h:2688h+2688] with
      10.75KB descriptors, alternating the sync/scalar HWDGE rings.
"""

import numpy as np

NV = 10000
C_TOTAL = 42000
KLIT = 3
B = 1024
NCORES = 8
C_CORE = C_TOTAL // NCORES     # 5250
GROUPS = 8                     # Q7 cores / 16-partition groups
C_PAD = 5376                   # padded clauses per core
H = 2                          # column chunks per core
C_CHUNK = C_PAD // H           # 2688 output cols per chunk
CPG = C_CHUNK // GROUPS        # 336 clauses per (group, chunk)
LPC = CPG * KLIT               # 1008 literals per (group, chunk)
LPC_PAD = 1024                 # padded to a 4B-aligned idx-col count
COLS_H = LPC_PAD // 16         # 64 idx cols per chunk (128B aligned)
IDX_COLS = H * COLS_H          # 128

_CACHE = {}


def _build():
    import concourse.bass as bass
    import concourse.tile as tile
    from concourse import bacc, mybir
    from contextlib import ExitStack

    f32 = mybir.dt.float32
    AF = mybir.ActivationFunctionType
    OP = mybir.AluOpType

    nc = bacc.Bacc("TRN2", target_bir_lowering=False, debug=False,
                   num_devices=NCORES)
    emb_d = nc.dram_tensor("emb", [1, NV], f32, kind="ExternalInput")
    idx_d = nc.dram_tensor("idxw", [128, IDX_COLS], mybir.dt.int16,
                           kind="ExternalInput")
    out_d = nc.dram_tensor("out", [B, C_PAD], f32, kind="ExternalOutput")

    with tile.TileContext(nc) as tc, ExitStack() as ctx:
        const = ctx.enter_context(tc.tile_pool(name="const", bufs=1))
        work = ctx.enter_context(tc.tile_pool(name="work", bufs=2))
        dpool = ctx.enter_context(
            tc.tile_pool(name="dram", bufs=2, space="DRAM"))

        idx_sb = const.tile([128, IDX_COLS], mybir.dt.int16)
        nc.sync.dma_start(out=idx_sb[:], in_=idx_d[:, :])

        raw = const.tile([128, NV], f32)
        tab = const.tile([128, 2 * NV], f32)
        half = NV // 2
        rings = [nc.sync, nc.scalar]
        for c in range(2):
            sl = slice(c * half, (c + 1) * half)
            rings[c].dma_start(
                out=raw[:, sl],
                in_=bass.AP(tensor=emb_d, offset=c * half,
                            ap=[[0, 128], [1, half]]))
        for c in range(2):
            sl = slice(c * half, (c + 1) * half)
            xs = slice(NV + c * half, NV + (c + 1) * half)
            ns = slice(c * half, (c + 1) * half)
            nc.scalar.activation(tab[:, xs], raw[:, sl], AF.Sigmoid)
            # 1 - x on DVE, overlaps ACT of the next half
            nc.vector.tensor_scalar(tab[:, ns], tab[:, xs], -1.0, 1.0,
                                    OP.mult, OP.add)

        for h in range(H):
            z = work.tile([128, LPC_PAD], f32, tag="z")
            nc.gpsimd.ap_gather(
                z[:], tab[:], idx_sb[:, h * COLS_H:(h + 1) * COLS_H],
                channels=128, num_elems=2 * NV, d=1, num_idxs=LPC_PAD)

            p01 = work.tile([128, CPG], f32, tag="p01")
            nc.vector.tensor_tensor(p01[:], z[:, 0:LPC:3], z[:, 1:LPC:3],
                                    OP.mult)
            r = work.tile([128, CPG], f32, tag="r")
            # r = ((p01 * -1) * z2) + 1 = 1 - z0 z1 z2
            nc.vector.scalar_tensor_tensor(r[:], p01[:], -1.0,
                                           z[:, 2:LPC:3], OP.mult, OP.mult)
            nc.vector.tensor_scalar_add(r[:], r[:], 1.0)

            # assemble the chunk's clause row into DRAM scratch: one
            # partition per group (partition 16g), 8 descriptors
            rap = r[:]
            pstride = rap.ap[0][0]
            row_t = dpool.tile([1, C_CHUNK], f32, tag="rowscratch")
            rowap = row_t[:]
            asm_src = bass.AP(tensor=rap.tensor, offset=rap.offset,
                              ap=[[pstride * 16, 8], [1, CPG]])
            asm_dst = bass.AP(tensor=rowap.tensor, offset=rowap.offset,
                              ap=[[CPG, 8], [1, CPG]])
            nc.gpsimd.dma_start(out=asm_dst, in_=asm_src)

            # broadcast-read the row chunk into all 128 partitions
            bcast = work.tile([128, C_CHUNK], f32, tag="bcast")
            nc.gpsimd.dma_start(
                out=bcast[:],
                in_=bass.AP(tensor=rowap.tensor, offset=rowap.offset,
                            ap=[[0, 128], [1, C_CHUNK]]))

            # 8 row-block output DMAs, 128 rows each, 10.75KB descriptors
            ring = rings[h % 2]
            bap = bcast[:]
            for blk in range(8):
                dst = bass.AP(tensor=out_d,
                              offset=blk * 128 * C_PAD + h * C_CHUNK,
                              ap=[[C_PAD, 128], [1, C_CHUNK]])
                ring.dma_start(out=dst, in_=bap)
    nc.compile()
    return nc


def _prep_indices(clause_idx, clause_sign):
    """Per-core wrapped int16 combined-index arrays [128, IDX_COLS].

    Literal order per group g: chunk-major — for chunk h, group g owns
    core clauses [C_CHUNK*h + CPG*g, C_CHUNK*h + CPG*(g+1)).
    """
    idx2 = clause_idx.astype(np.int32) + NV * (clause_sign <= 0.0)
    idx2 = idx2.astype(np.int16)
    per_core = []
    for c in range(NCORES):
        cl = idx2[c * C_CORE:(c + 1) * C_CORE]            # [5250, 3]
        buf = np.zeros((C_PAD, KLIT), dtype=np.int16)
        buf[:cl.shape[0]] = cl
        # [H, GROUPS, LPC] -> pad each (group, chunk) block to LPC_PAD
        gl = buf.reshape(H, GROUPS, LPC)
        glp = np.zeros((H, GROUPS, LPC_PAD), dtype=np.int16)
        glp[:, :, :LPC] = gl
        # group g's stream = concat over h  -> [GROUPS, H*LPC_PAD]
        gs = glp.transpose(1, 0, 2).reshape(GROUPS, H * LPC_PAD)
        # wrap: literal j at partition 16g + j%16, col j//16
        w = (gs.reshape(GROUPS, IDX_COLS, 16)
               .transpose(0, 2, 1)
               .reshape(128, IDX_COLS))
        per_core.append(np.ascontiguousarray(w))
    return per_core


def _ensure_ntff_hook():
    """The agent image lacks antenv.axon_hooks; synthesize it so
    run_bass_kernel_spmd(trace=True) can capture NTFF profiles."""
    import sys, types
    try:
        from antenv import axon_hooks  # noqa: F401
        return
    except ImportError:
        pass
    m = types.ModuleType("antenv.axon_hooks")
    _hook = [None]
    m.set_axon_ntff_profile_hook = lambda h: _hook.__setitem__(0, h)
    m.get_axon_ntff_profile_hook = lambda: _hook[0]
    sys.modules["antenv.axon_hooks"] = m
    import antenv
    antenv.axon_hooks = m
    from trn_agent_boot.trn_boot import _ntff_profile_via_ctypes
    m.set_axon_ntff_profile_hook(
        _ntff_profile_via_ctypes("/opt/axon/libaxon_pjrt.so"))


def _run(emb, idx_cores, trace=False):
    from concourse.bass_utils import run_bass_kernel_spmd
    if trace:
        _ensure_ntff_hook()
    if "prog" not in _CACHE:
        _CACHE["prog"] = _build()
    nc = _CACHE["prog"]
    in_maps = [{"emb": emb, "idxw": idx_cores[c]} for c in range(NCORES)]
    return run_bass_kernel_spmd(nc, in_maps, list(range(NCORES)),
                                trace=trace)


def kernel(input_idx=None, emb_weight=None, clause_idx=None,
           clause_sign=None, _trace=False, _want_results=False):
    emb = np.ascontiguousarray(np.asarray(emb_weight, dtype=np.float32))
    cidx = np.asarray(clause_idx, dtype=np.int32)
    csgn = np.asarray(clause_sign, dtype=np.float32)
    idx_cores = _prep_indices(cidx, csgn)
    res = _run(emb, idx_cores, trace=_trace)
    full = np.empty((B, C_TOTAL), dtype=np.float32)
    for c in range(NCORES):
        full[:, c * C_CORE:(c + 1) * C_CORE] = \
            res.results[c]["out"][:, :C_CORE]
    if _want_results:
        return full, res
    return full


# revision 16
# speedup vs baseline: 2.2002x; 1.2732x over previous
"""Trainium2 Bass kernel: batched soft 3-SAT circuit evaluation.

out[b, c] = 1 - prod_k z[c,k],  z = (sign>0 ? 1-x : x)[idx],
x = sigmoid(emb[0]).  Every batch row is identical (input_idx is all
zeros, the embedding has a single row, and jnp.take clamps OOB), so the
device computes each clause result once and broadcast-writes the rows.

Sharding: clauses split across 8 NeuronCores (5250 each, padded 5376).
Host work is index-layout prep only (fold sign into a combined table
index, pad, order literals chunk-major, wrap into the 16-partition
GPSIMD gather layout) plus concatenation of per-core outputs.

Per-core device pipeline (H = 4 column chunks of 1344 cols):
  prologue (4 col-quarters, two HWDGE rings): broadcast-load emb row
    into raw[128, NV]; ACT sigmoid -> x table half; DVE (x*-1)+1 ->
    1-x table half.  Combined table tab[128, 2*NV].
  per chunk h:
    - GPSIMD ap_gather: z[128, 512] literals (8 Q7 groups x 168 clauses)
    - DVE: r = 1 - z0*z1*z2  [128, 168] (replicated within each
      16-partition group)
    - PE: per group g a [K=16]x[M=128]x[N=168] matmul with lhsT=1/16
      broadcasts group g's row into all 128 partitions of PSUM (bitwise
      exact: sum of 16 identical values * 1/16)
    - ACT: copy PSUM -> SBUF bcast tile [128, 8*168]
    - 8 row-block DMAs bcast -> out[128b:128b+128, 1344h:1344h+1344]
      (5.4KB descriptors), alternating the sync/scalar HWDGE rings.
"""

import numpy as np

NV = 10000
C_TOTAL = 42000
KLIT = 3
B = 1024
NCORES = 8
C_CORE = C_TOTAL // NCORES     # 5250
GROUPS = 8                     # Q7 cores / 16-partition groups
C_PAD = 5376                   # padded clauses per core
H = 4                          # column chunks per core
C_CHUNK = C_PAD // H           # 1344 output cols per chunk
CPG = C_CHUNK // GROUPS        # 168 clauses per (group, chunk)
LPC = CPG * KLIT               # 504 literals per (group, chunk)
LPC_PAD = 512                  # padded to a 4B-aligned idx-col count
COLS_H = LPC_PAD // 16         # 32 idx cols per chunk (64B aligned)
IDX_COLS = H * COLS_H          # 128
PBLK = 256                     # PSUM cols reserved per group block

_CACHE = {}


def _build():
    import concourse.bass as bass
    import concourse.tile as tile
    from concourse import bacc, mybir
    from contextlib import ExitStack

    f32 = mybir.dt.float32
    AF = mybir.ActivationFunctionType
    OP = mybir.AluOpType

    nc = bacc.Bacc("TRN2", target_bir_lowering=False, debug=False,
                   num_devices=NCORES)
    emb_d = nc.dram_tensor("emb", [1, NV], f32, kind="ExternalInput")
    idx_d = nc.dram_tensor("idxw", [128, IDX_COLS], mybir.dt.int16,
                           kind="ExternalInput")
    out_d = nc.dram_tensor("out", [B, C_PAD], f32, kind="ExternalOutput")

    with tile.TileContext(nc) as tc, ExitStack() as ctx:
        const = ctx.enter_context(tc.tile_pool(name="const", bufs=1))
        work = ctx.enter_context(tc.tile_pool(name="work", bufs=2))
        psum = ctx.enter_context(
            tc.tile_pool(name="psum", bufs=2, space="PSUM"))

        idx_sb = const.tile([128, IDX_COLS], mybir.dt.int16)
        nc.sync.dma_start(out=idx_sb[:], in_=idx_d[:, :])

        # selector E[:, g, :]: E[k, g, m] = 1/16 iff k//16 == g; matmul
        # with it averages each group's 16 identical partition rows into
        # all 128 output partitions (bitwise exact).
        sel = const.tile([128, GROUPS, 128], f32)
        nc.vector.memset(sel[:], 1.0 / 16.0)
        # keep 1/16 only where 0 <= p - 16g <= 15, i.e. g == p//16
        nc.gpsimd.affine_select(sel[:, :, :], sel[:, :, :],
                                pattern=[[-16, GROUPS], [0, 128]],
                                compare_op=OP.is_ge, fill=0.0,
                                base=0, channel_multiplier=1)
        nc.gpsimd.affine_select(sel[:, :, :], sel[:, :, :],
                                pattern=[[16, GROUPS], [0, 128]],
                                compare_op=OP.is_ge, fill=0.0,
                                base=15, channel_multiplier=-1)

        raw = const.tile([128, NV], f32)
        tab = const.tile([128, 2 * NV], f32)
        rings = [nc.sync, nc.scalar]
        NQ = 4
        q = NV // NQ
        for c in range(NQ):
            rings[c % 2].dma_start(
                out=raw[:, c * q:(c + 1) * q],
                in_=bass.AP(tensor=emb_d, offset=c * q,
                            ap=[[0, 128], [1, q]]))
        for c in range(NQ):
            sl = slice(c * q, (c + 1) * q)
            xs = slice(NV + c * q, NV + (c + 1) * q)
            nc.scalar.activation(tab[:, xs], raw[:, sl], AF.Sigmoid)
            # 1 - x on DVE, overlaps ACT of the next quarter
            nc.vector.tensor_scalar(tab[:, sl], tab[:, xs], -1.0, 1.0,
                                    OP.mult, OP.add)

        for h in range(H):
            z = work.tile([128, LPC_PAD], f32, tag="z")
            nc.gpsimd.ap_gather(
                z[:], tab[:], idx_sb[:, h * COLS_H:(h + 1) * COLS_H],
                channels=128, num_elems=2 * NV, d=1, num_idxs=LPC_PAD)

            p01 = work.tile([128, CPG], f32, tag="p01")
            nc.vector.tensor_tensor(p01[:], z[:, 0:LPC:3], z[:, 1:LPC:3],
                                    OP.mult)
            r = work.tile([128, CPG], f32, tag="r")
            # r = ((p01 * -1) * z2) + 1 = 1 - z0 z1 z2
            nc.vector.scalar_tensor_tensor(r[:], p01[:], -1.0,
                                           z[:, 2:LPC:3], OP.mult, OP.mult)
            nc.vector.tensor_scalar_add(r[:], r[:], 1.0)

            # PE broadcast: group g's (16-replicated) row -> all 128
            # partitions.  sum over the 16 identical values * 1/16 is
            # bitwise exact.
            P = psum.tile([128, GROUPS, PBLK], f32, tag="P")
            for g in range(GROUPS):
                nc.tensor.matmul(P[:, g, 0:CPG], sel[:, g, :], r[:, :],
                                 start=True, stop=True)
            bcast = work.tile([128, GROUPS, CPG], f32, tag="bcast")
            nc.scalar.activation(bcast[:, :, :], P[:, :, 0:CPG], AF.Copy)

            # 8 row-block output DMAs, 128 rows each, 5.4KB descriptors
            ring = rings[h % 2]
            bap = bass.AP(tensor=bcast[:].tensor, offset=bcast[:].offset,
                          ap=[[bcast[:].ap[0][0], 128], [1, C_CHUNK]])
            for blk in range(8):
                dst = bass.AP(tensor=out_d,
                              offset=blk * 128 * C_PAD + h * C_CHUNK,
                              ap=[[C_PAD, 128], [1, C_CHUNK]])
                ring.dma_start(out=dst, in_=bap)
    nc.compile()
    return nc


def _prep_indices(clause_idx, clause_sign):
    """Per-core wrapped int16 combined-index arrays [128, IDX_COLS].

    Literal order per group g: chunk-major — for chunk h, group g owns
    core clauses [C_CHUNK*h + CPG*g, C_CHUNK*h + CPG*(g+1)), padded to
    LPC_PAD literals per (group, chunk) block.
    """
    idx2 = clause_idx.astype(np.int32) + NV * (clause_sign <= 0.0)
    idx2 = idx2.astype(np.int16)
    per_core = []
    for c in range(NCORES):
        cl = idx2[c * C_CORE:(c + 1) * C_CORE]            # [5250, 3]
        buf = np.zeros((C_PAD, KLIT), dtype=np.int16)
        buf[:cl.shape[0]] = cl
        gl = buf.reshape(H, GROUPS, LPC)
        glp = np.zeros((H, GROUPS, LPC_PAD), dtype=np.int16)
        glp[:, :, :LPC] = gl
        # group g's stream = concat over h  -> [GROUPS, H*LPC_PAD]
        gs = glp.transpose(1, 0, 2).reshape(GROUPS, H * LPC_PAD)
        # wrap: literal j at partition 16g + j%16, col j//16
        w = (gs.reshape(GROUPS, IDX_COLS, 16)
               .transpose(0, 2, 1)
               .reshape(128, IDX_COLS))
        per_core.append(np.ascontiguousarray(w))
    return per_core


def _ensure_ntff_hook():
    """The agent image lacks antenv.axon_hooks; synthesize it so
    run_bass_kernel_spmd(trace=True) can capture NTFF profiles."""
    import sys, types
    try:
        from antenv import axon_hooks  # noqa: F401
        return
    except ImportError:
        pass
    m = types.ModuleType("antenv.axon_hooks")
    _hook = [None]
    m.set_axon_ntff_profile_hook = lambda h: _hook.__setitem__(0, h)
    m.get_axon_ntff_profile_hook = lambda: _hook[0]
    sys.modules["antenv.axon_hooks"] = m
    import antenv
    antenv.axon_hooks = m
    from trn_agent_boot.trn_boot import _ntff_profile_via_ctypes
    m.set_axon_ntff_profile_hook(
        _ntff_profile_via_ctypes("/opt/axon/libaxon_pjrt.so"))


def _run(emb, idx_cores, trace=False):
    from concourse.bass_utils import run_bass_kernel_spmd
    if trace:
        _ensure_ntff_hook()
    if "prog" not in _CACHE:
        _CACHE["prog"] = _build()
    nc = _CACHE["prog"]
    in_maps = [{"emb": emb, "idxw": idx_cores[c]} for c in range(NCORES)]
    return run_bass_kernel_spmd(nc, in_maps, list(range(NCORES)),
                                trace=trace)


def kernel(input_idx=None, emb_weight=None, clause_idx=None,
           clause_sign=None, _trace=False, _want_results=False):
    emb = np.ascontiguousarray(np.asarray(emb_weight, dtype=np.float32))
    cidx = np.asarray(clause_idx, dtype=np.int32)
    csgn = np.asarray(clause_sign, dtype=np.float32)
    idx_cores = _prep_indices(cidx, csgn)
    res = _run(emb, idx_cores, trace=_trace)
    full = np.empty((B, C_TOTAL), dtype=np.float32)
    for c in range(NCORES):
        full[:, c * C_CORE:(c + 1) * C_CORE] = \
            res.results[c]["out"][:, :C_CORE]
    if _want_results:
        return full, res
    return full


# revision 19
# speedup vs baseline: 2.2342x; 1.0155x over previous
"""Trainium2 Bass kernel: batched soft 3-SAT circuit evaluation.

out[b, c] = 1 - prod_k z[c,k],  z = (sign>0 ? 1-x : x)[idx],
x = sigmoid(emb[0]).  Every batch row is identical (input_idx is all
zeros, the embedding has a single row, and jnp.take clamps OOB), so the
device computes each clause result once and broadcast-writes the rows.

Sharding: clauses split across 8 NeuronCores (5250 each, padded 5376).
Host work is index-layout prep only (fold sign into a combined table
index, pad, order literals chunk-major, wrap into the 16-partition
GPSIMD gather layout) plus concatenation of per-core outputs.

Per-core device pipeline (H = 4 column chunks of 1344 cols):
  prologue (4 col-quarters, two HWDGE rings): broadcast-load emb row
    into raw[128, NV]; ACT sigmoid -> x table half; DVE (x*-1)+1 ->
    1-x table half.  Combined table tab[128, 2*NV].
  per chunk h:
    - GPSIMD ap_gather: z[128, 512] literals (8 Q7 groups x 168 clauses)
    - DVE: r = 1 - z0*z1*z2  [128, 168] (replicated within each
      16-partition group)
    - PE: per group g a [K=16]x[M=128]x[N=168] matmul with lhsT=1/16
      broadcasts group g's row into all 128 partitions of PSUM (bitwise
      exact: sum of 16 identical values * 1/16)
    - ACT: copy PSUM -> SBUF bcast tile [128, 8*168]
    - 8 row-block DMAs bcast -> out[128b:128b+128, 1344h:1344h+1344]
      (5.4KB descriptors), alternating the sync/scalar HWDGE rings.
"""

import numpy as np

NV = 10000
C_TOTAL = 42000
KLIT = 3
B = 1024
NCORES = 8
C_CORE = C_TOTAL // NCORES     # 5250
GROUPS = 8                     # Q7 cores / 16-partition groups
C_PAD = 5376                   # padded clauses per core
H = 4                          # column chunks per core
C_CHUNK = C_PAD // H           # 1344 output cols per chunk
CPG = C_CHUNK // GROUPS        # 168 clauses per (group, chunk)
LPC = CPG * KLIT               # 504 literals per (group, chunk)
LPC_PAD = 512                  # padded to a 4B-aligned idx-col count
COLS_H = LPC_PAD // 16         # 32 idx cols per chunk (64B aligned)
IDX_COLS = H * COLS_H          # 128
PBLK = 256                     # PSUM cols reserved per group block

_CACHE = {}


def _build():
    import concourse.bass as bass
    import concourse.tile as tile
    from concourse import bacc, mybir
    from contextlib import ExitStack

    f32 = mybir.dt.float32
    AF = mybir.ActivationFunctionType
    OP = mybir.AluOpType

    nc = bacc.Bacc("TRN2", target_bir_lowering=False, debug=False,
                   num_devices=NCORES)
    emb_d = nc.dram_tensor("emb", [1, NV], f32, kind="ExternalInput")
    idx_d = nc.dram_tensor("idxw", [128, IDX_COLS], mybir.dt.int16,
                           kind="ExternalInput")
    out_d = nc.dram_tensor("out", [B, C_PAD], f32, kind="ExternalOutput")

    with tile.TileContext(nc) as tc, ExitStack() as ctx:
        const = ctx.enter_context(tc.tile_pool(name="const", bufs=1))
        work = ctx.enter_context(tc.tile_pool(name="work", bufs=2))
        psum = ctx.enter_context(
            tc.tile_pool(name="psum", bufs=2, space="PSUM"))

        idx_sb = const.tile([128, IDX_COLS], mybir.dt.int16)

        # selector E[:, g, :]: E[k, g, m] = 1/16 iff k//16 == g; matmul
        # with it averages each group's 16 identical partition rows into
        # all 128 output partitions (bitwise exact).
        sel = const.tile([128, GROUPS, 128], f32)
        nc.vector.memset(sel[:], 1.0 / 16.0)
        # keep 1/16 only where 0 <= p - 16g <= 15, i.e. g == p//16
        nc.gpsimd.affine_select(sel[:, :, :], sel[:, :, :],
                                pattern=[[-16, GROUPS], [0, 128]],
                                compare_op=OP.is_ge, fill=0.0,
                                base=0, channel_multiplier=1)
        nc.gpsimd.affine_select(sel[:, :, :], sel[:, :, :],
                                pattern=[[16, GROUPS], [0, 128]],
                                compare_op=OP.is_ge, fill=0.0,
                                base=15, channel_multiplier=-1)

        raw = const.tile([128, NV], f32)
        tab = const.tile([128, 2 * NV], f32)
        rings = [nc.sync, nc.scalar]
        NQ = 4
        q = NV // NQ
        # first quarter alone at the head of the sync ring so ACT can
        # start early; everything else afterwards / on the other ring
        for c in range(NQ):
            rings[c % 2].dma_start(
                out=raw[:, c * q:(c + 1) * q],
                in_=bass.AP(tensor=emb_d, offset=c * q,
                            ap=[[0, 128], [1, q]]))
        nc.scalar.dma_start(out=idx_sb[:], in_=idx_d[:, :])
        for c in range(NQ):
            sl = slice(c * q, (c + 1) * q)
            xs = slice(NV + c * q, NV + (c + 1) * q)
            nc.scalar.activation(tab[:, xs], raw[:, sl], AF.Sigmoid)
            # 1 - x on DVE, overlaps ACT of the next quarter
            nc.vector.tensor_scalar(tab[:, sl], tab[:, xs], -1.0, 1.0,
                                    OP.mult, OP.add)

        for h in range(H):
            z = work.tile([128, LPC_PAD], f32, tag="z")
            nc.gpsimd.ap_gather(
                z[:], tab[:], idx_sb[:, h * COLS_H:(h + 1) * COLS_H],
                channels=128, num_elems=2 * NV, d=1, num_idxs=LPC_PAD)

            p01 = work.tile([128, CPG], f32, tag="p01")
            nc.vector.tensor_tensor(p01[:], z[:, 0:LPC:3], z[:, 1:LPC:3],
                                    OP.mult)
            r = work.tile([128, CPG], f32, tag="r")
            # r = ((p01 * -1) * z2) + 1 = 1 - z0 z1 z2
            nc.vector.scalar_tensor_tensor(r[:], p01[:], -1.0,
                                           z[:, 2:LPC:3], OP.mult, OP.mult)
            nc.vector.tensor_scalar_add(r[:], r[:], 1.0)

            # PE broadcast: group g's (16-replicated) row -> all 128
            # partitions.  sum over the 16 identical values * 1/16 is
            # bitwise exact.
            P = psum.tile([128, GROUPS, PBLK], f32, tag="P")
            for g in range(GROUPS):
                nc.tensor.matmul(P[:, g, 0:CPG], sel[:, g, :], r[:, :],
                                 start=True, stop=True)
            bcast = work.tile([128, GROUPS, CPG], f32, tag="bcast")
            nc.scalar.activation(bcast[:, :, :], P[:, :, 0:CPG], AF.Copy)

            # 8 row-block output DMAs, 128 rows each, 5.4KB descriptors,
            # spread across both HWDGE rings
            bap = bass.AP(tensor=bcast[:].tensor, offset=bcast[:].offset,
                          ap=[[bcast[:].ap[0][0], 128], [1, C_CHUNK]])
            for blk in range(8):
                dst = bass.AP(tensor=out_d,
                              offset=blk * 128 * C_PAD + h * C_CHUNK,
                              ap=[[C_PAD, 128], [1, C_CHUNK]])
                rings[blk % 2].dma_start(out=dst, in_=bap)
    nc.compile()
    return nc


def _prep_indices(clause_idx, clause_sign):
    """Per-core wrapped int16 combined-index arrays [128, IDX_COLS].

    Literal order per group g: chunk-major — for chunk h, group g owns
    core clauses [C_CHUNK*h + CPG*g, C_CHUNK*h + CPG*(g+1)), padded to
    LPC_PAD literals per (group, chunk) block.
    """
    idx2 = clause_idx.astype(np.int32) + NV * (clause_sign <= 0.0)
    idx2 = idx2.astype(np.int16)
    per_core = []
    for c in range(NCORES):
        cl = idx2[c * C_CORE:(c + 1) * C_CORE]            # [5250, 3]
        buf = np.zeros((C_PAD, KLIT), dtype=np.int16)
        buf[:cl.shape[0]] = cl
        gl = buf.reshape(H, GROUPS, LPC)
        glp = np.zeros((H, GROUPS, LPC_PAD), dtype=np.int16)
        glp[:, :, :LPC] = gl
        # group g's stream = concat over h  -> [GROUPS, H*LPC_PAD]
        gs = glp.transpose(1, 0, 2).reshape(GROUPS, H * LPC_PAD)
        # wrap: literal j at partition 16g + j%16, col j//16
        w = (gs.reshape(GROUPS, IDX_COLS, 16)
               .transpose(0, 2, 1)
               .reshape(128, IDX_COLS))
        per_core.append(np.ascontiguousarray(w))
    return per_core


def _ensure_ntff_hook():
    """The agent image lacks antenv.axon_hooks; synthesize it so
    run_bass_kernel_spmd(trace=True) can capture NTFF profiles."""
    import sys, types
    try:
        from antenv import axon_hooks  # noqa: F401
        return
    except ImportError:
        pass
    m = types.ModuleType("antenv.axon_hooks")
    _hook = [None]
    m.set_axon_ntff_profile_hook = lambda h: _hook.__setitem__(0, h)
    m.get_axon_ntff_profile_hook = lambda: _hook[0]
    sys.modules["antenv.axon_hooks"] = m
    import antenv
    antenv.axon_hooks = m
    from trn_agent_boot.trn_boot import _ntff_profile_via_ctypes
    m.set_axon_ntff_profile_hook(
        _ntff_profile_via_ctypes("/opt/axon/libaxon_pjrt.so"))


def _run(emb, idx_cores, trace=False):
    from concourse.bass_utils import run_bass_kernel_spmd
    if trace:
        _ensure_ntff_hook()
    if "prog" not in _CACHE:
        _CACHE["prog"] = _build()
    nc = _CACHE["prog"]
    in_maps = [{"emb": emb, "idxw": idx_cores[c]} for c in range(NCORES)]
    return run_bass_kernel_spmd(nc, in_maps, list(range(NCORES)),
                                trace=trace)


def kernel(input_idx=None, emb_weight=None, clause_idx=None,
           clause_sign=None, _trace=False, _want_results=False):
    emb = np.ascontiguousarray(np.asarray(emb_weight, dtype=np.float32))
    cidx = np.asarray(clause_idx, dtype=np.int32)
    csgn = np.asarray(clause_sign, dtype=np.float32)
    idx_cores = _prep_indices(cidx, csgn)
    res = _run(emb, idx_cores, trace=_trace)
    full = np.empty((B, C_TOTAL), dtype=np.float32)
    for c in range(NCORES):
        full[:, c * C_CORE:(c + 1) * C_CORE] = \
            res.results[c]["out"][:, :C_CORE]
    if _want_results:
        return full, res
    return full
